# revision 1
# baseline (speedup 1.0000x reference)
"""DeepFM forward (embedding gather + FM + MLP) on 8 Trainium2 NeuronCores.

Strategy: data-parallel over the batch (2048 rows/core), embedding tables
replicated per core (input staging is off the measured path, no collectives).

Per core:
  - emb2 [F,V,16] and emb1 [F,V,1] are packed host-side into one bf16 table
    [F*V, 18] (36 B word-aligned rows); lookup indices become global rows
    f*V + X[b,f].
  - SWDGE indirect DMA gathers the rows. The HW honours exactly one index
    per partition per op, so each op fetches 128 rows (one batch-tile x one
    field) into its slice of a batch-major [128, 4*468] staging tile; 416
    ops per core, which is the kernel's bottleneck (~1.1 us/op of Q7
    descriptor generation).
  - PE transposes 128x128 blocks into feature-major [468, 512] tiles.
  - BatchNorm (eval mode) is folded into W1/W2 host-side; the MLP runs as
    bf16 matmuls (fp32 PSUM accumulate) with ReLU+bias fused in ScalarE
    activations. fp32/f32r matmuls are avoided: they lower to the S3_LW
    struct, which has a single sync-wait slot and fails walrus codegen
    under Tile's multi-wait schedules.
  - FM terms come from matmuls with constant selector matrices (first-order
    sum rides the A-matmul at output partition 32 to satisfy the SBUF
    base-partition rule); everything accumulates into one [1, 512] PSUM
    tile; Sigmoid+b3 fused at the end.
"""

import os
import sys

sys.path.insert(0, "/opt/trn_rl_repo")
os.environ.setdefault("MYCRO_LOCAL_CACHE", "1")

import numpy as np
from ml_dtypes import bfloat16 as np_bf16

import concourse.bass as bass
import concourse.bacc as bacc
import concourse.tile as tile
from concourse import mybir
from concourse.bass_utils import run_bass_kernel_spmd
from concourse.masks import make_identity

# Problem dims (hardcoded; kernel.py must be self-contained).
B, F, V, D = 16384, 26, 100000, 16
H1, H2 = 256, 128
EPS = 1e-5

NCORES = 8
NB = B // NCORES          # 2048 batch rows per core
P = 128
BT = NB // P              # 16 batch tiles per core
TR = D + 2                # 18 bf16 per packed table row: 16 emb2 + emb1 + pad
                          # (36 B, 4-byte aligned: a 34 B row puts odd rows on
                          # a 2-byte boundary, which HW gather mishandles)
NF = F * TR               # 468 feature rows (transposed layout)
FCH = [128, 128, 128, 84]  # feature-chunk partition counts (sum = 468)
TS = 48                   # A-matmul output rows: s_d in 0..15, fm1 at 32
# packed f32r weight tensor column offsets
WC_W1 = 0                 # 4 chunks x 256
WC_A = 1024               # 4 chunks x 48
WC_U = WC_A + 4 * TS      # 4 chunks x 1
WC_W2 = WC_U + 4          # 2 chunks x 128
WC_W3 = WC_W2 + 256       # 1
WC_FIN = WC_W3 + 1        # 1
WRC = WC_FIN + 1          # total packed f32r columns
NCHN = 4                  # N-chunks per core
NN = NB // NCHN           # 512 batch columns per N-chunk
BT_N = NN // P            # 4 batch tiles per N-chunk

F32 = mybir.dt.float32
BF16 = mybir.dt.bfloat16
I32 = mybir.dt.int32

TRACE = os.environ.get("BASS_KERNEL_TRACE", "0") == "1"
LAST_RESULTS = None

_NC_CACHE = None


def _build_nc():
    # Bacc + .compile() (not plain Bass): compile() runs
    # generate_event_semaphores, which splits multi-sem waits to satisfy
    # the TRN2 1-wait-per-instruction ISA constraint.
    nc = bacc.Bacc(
        "TRN2", target_bir_lowering=False, debug=False, num_devices=NCORES
    )

    idx = nc.dram_tensor("idx", [P, BT * F], I32, kind="ExternalInput")
    table = nc.dram_tensor("table", [F * V, TR], BF16, kind="ExternalInput")
    wpack_r = nc.dram_tensor("wpack_r", [P, WRC], BF16, kind="ExternalInput")
    wpack_f = nc.dram_tensor("wpack_f", [P, 4], F32, kind="ExternalInput")
    out = nc.dram_tensor("out", [1, NB], F32, kind="ExternalOutput")

    AF = mybir.ActivationFunctionType

    with tile.TileContext(nc) as tc:
        with (
            tc.tile_pool(name="const", bufs=1) as const,
            tc.tile_pool(name="gat", bufs=4) as gat,
            tc.tile_pool(name="et", bufs=2) as etp,
            tc.tile_pool(name="sq", bufs=2) as sqp,
            tc.tile_pool(name="h1", bufs=2) as h1p,
            tc.tile_pool(name="h2", bufs=2) as h2p,
            tc.tile_pool(name="ssq", bufs=2) as ssqp,
            tc.tile_pool(name="ob", bufs=2) as obp,
            tc.tile_pool(name="tp", bufs=2, space="PSUM") as tpp,
            tc.tile_pool(name="p1", bufs=2, space="PSUM") as p1p,
            tc.tile_pool(name="p2", bufs=1, space="PSUM") as p2p,
            tc.tile_pool(name="ps", bufs=1, space="PSUM") as psp,
            tc.tile_pool(name="pl", bufs=1, space="PSUM") as plp,
        ):
            # ---- constants / weights to SBUF (3 DMAs total: fewer DMA
            # lanes keeps per-instruction sync-wait counts inside the ISA
            # wait-slot limit) ----
            idx_t = const.tile([P, BT * F], I32)
            nc.sync.dma_start(out=idx_t[:], in_=idx[:])
            wr = const.tile([P, WRC], BF16, tag="wr")
            nc.sync.dma_start(out=wr[:], in_=wpack_r[:])
            wf = const.tile([P, 4], F32, tag="wf")
            nc.sync.dma_start(out=wf[:], in_=wpack_f[:])

            w1_t = [wr[:, WC_W1 + c * H1: WC_W1 + (c + 1) * H1] for c in range(4)]
            a_t = [wr[:, WC_A + c * TS: WC_A + (c + 1) * TS] for c in range(4)]
            u_t = [wr[:, WC_U + c: WC_U + c + 1] for c in range(4)]
            w2_t = [wr[:, WC_W2 + k * H2: WC_W2 + (k + 1) * H2] for k in range(2)]
            w3_t = wr[:, WC_W3: WC_W3 + 1]
            wfin_t = wr[:TS, WC_FIN: WC_FIN + 1]
            c1_t = wf[:, 0:2]
            c2_t = wf[:, 2:3]
            b3_t = wf[0:1, 3:4]

            ident = const.tile([P, P], BF16, tag="ident")
            make_identity(nc, ident[:])

            # ---- main loop over N-chunks of 512 batch columns ----
            for n in range(NCHN):
                # Gather 512 batch rows x 26 tables -> [128, 4*468] batch-major.
                # HW indirect DMA honours ONE index per partition (it streams
                # the dest's free bytes consecutively from that row), so each
                # op gathers 128 rows: one (batch-tile, field) pair per op,
                # landing at its slice of the staging tile.
                g = gat.tile([P, BT_N * NF], BF16, tag="g")
                for o in range(BT_N * F):
                    col = n * (BT_N * F) + o
                    nc.gpsimd.indirect_dma_start(
                        out=g[:, o * TR:(o + 1) * TR],
                        out_offset=None,
                        in_=table[:],
                        in_offset=bass.IndirectOffsetOnAxis(
                            ap=idx_t[:, col:col + 1],
                            axis=0,
                        ),
                    )

                # Transpose to feature-major eT chunks [FCH[c], 512].
                et = [etp.tile([P, NN], BF16, tag=f"et{c}", name=f"et{c}_{n}") for c in range(4)]
                for tl in range(BT_N):
                    for c in range(4):
                        ch = FCH[c]
                        col0 = tl * NF + c * P
                        pt = tpp.tile([P, P], BF16, tag="tp")
                        nc.tensor.transpose(
                            out=pt[:ch, :],
                            in_=g[:, col0:col0 + ch],
                            identity=ident[:],
                        )
                        nc.vector.tensor_copy(
                            out=et[c][:ch, tl * P:(tl + 1) * P],
                            in_=pt[:ch, :],
                        )

                # Squared copies for the FM second-order sum(e^2) term.
                sq = [sqp.tile([P, NN], BF16, tag=f"sq{c}", name=f"sq{c}_{n}") for c in range(4)]
                for c in range(4):
                    ch = FCH[c]
                    nc.vector.tensor_mul(
                        out=sq[c][:ch, :], in0=et[c][:ch, :], in1=et[c][:ch, :]
                    )

                # A-matmul: rows 0..15 = s_d (sum_f e2), row 16 = fm first order.
                ps = psp.tile([TS, NN], F32, tag="ps")
                for c in range(4):
                    ch = FCH[c]
                    nc.tensor.matmul(
                        out=ps[:],
                        lhsT=a_t[c][:ch, :],
                        rhs=et[c][:ch, :],
                        start=(c == 0),
                        stop=(c == 3),
                    )
                ssq = ssqp.tile([TS, NN], BF16, tag="ssq")
                nc.scalar.activation(ssq[:32, :], ps[:32, :], AF.Square)
                nc.scalar.copy(ssq[32:TS, :], ps[32:TS, :])

                # MLP layer 1: [442 -> 256], ReLU + folded-BN bias.
                h1 = [h1p.tile([P, NN], BF16, tag=f"h1_{m}", name=f"h1_{m}_{n}") for m in range(2)]
                for m in range(2):
                    p1 = p1p.tile([P, NN], F32, tag="p1")
                    for c in range(4):
                        ch = FCH[c]
                        nc.tensor.matmul(
                            out=p1[:],
                            lhsT=w1_t[c][:ch, m * P:(m + 1) * P],
                            rhs=et[c][:ch, :],
                            start=(c == 0),
                            stop=(c == 3),
                        )
                    nc.scalar.activation(
                        h1[m][:], p1[:], AF.Relu, bias=c1_t[:, m:m + 1]
                    )

                # MLP layer 2: [256 -> 128], ReLU + folded-BN bias.
                p2 = p2p.tile([P, NN], F32, tag="p2")
                for k in range(2):
                    nc.tensor.matmul(
                        out=p2[:],
                        lhsT=w2_t[k][:, :],
                        rhs=h1[k][:],
                        start=(k == 0),
                        stop=(k == 1),
                    )
                h2 = h2p.tile([P, NN], BF16, tag="h2")
                nc.scalar.activation(h2[:], p2[:], AF.Relu, bias=c2_t[:, 0:1])

                # Logits: W3.T@h2 - 0.5*sum(e^2) + 0.5*sum(s^2) + fm1, then
                # sigmoid(x + b3).
                pl = plp.tile([1, NN], F32, tag="pl")
                nc.tensor.matmul(
                    out=pl[:], lhsT=w3_t[:, :], rhs=h2[:],
                    start=True, stop=False,
                )
                for c in range(4):
                    ch = FCH[c]
                    nc.tensor.matmul(
                        out=pl[:],
                        lhsT=u_t[c][:ch, :],
                        rhs=sq[c][:ch, :],
                        start=False, stop=False,
                    )
                nc.tensor.matmul(
                    out=pl[:], lhsT=wfin_t[:, :], rhs=ssq[:],
                    start=False, stop=True,
                )
                ob = obp.tile([1, NN], F32, tag="ob")
                nc.scalar.activation(ob[:], pl[:], AF.Sigmoid, bias=b3_t[:, :])
                nc.sync.dma_start(out=out[0:1, n * NN:(n + 1) * NN], in_=ob[:])

    nc.compile()
    return nc


def _get_nc():
    global _NC_CACHE
    if _NC_CACHE is None:
        _NC_CACHE = _build_nc()
    return _NC_CACHE


TS_ = TS


def kernel(X_sparse, emb1, emb2, W1, b1, g1, be1, m1, v1,
           W2, b2, g2, be2, m2, v2, W3, b3):
    global LAST_RESULTS

    X_sparse = np.asarray(X_sparse)
    emb1 = np.asarray(emb1, np.float32)
    emb2 = np.asarray(emb2, np.float32)
    W1 = np.asarray(W1, np.float32)
    b1 = np.asarray(b1, np.float32)
    g1 = np.asarray(g1, np.float32)
    be1 = np.asarray(be1, np.float32)
    m1 = np.asarray(m1, np.float32)
    v1 = np.asarray(v1, np.float32)
    W2 = np.asarray(W2, np.float32)
    b2 = np.asarray(b2, np.float32)
    g2 = np.asarray(g2, np.float32)
    be2 = np.asarray(be2, np.float32)
    m2 = np.asarray(m2, np.float32)
    v2 = np.asarray(v2, np.float32)
    W3 = np.asarray(W3, np.float32)
    b3 = np.asarray(b3, np.float32)

    # Pack emb2 + emb1 into one bf16 gather table [F*V, 18] (36 B rows).
    table = np.zeros((F * V, TR), np_bf16)
    table[:, :D] = emb2.reshape(F * V, D).astype(np_bf16)
    table[:, D] = emb1.reshape(F * V).astype(np_bf16)

    # Fold eval-mode BatchNorm into the matmul weights/biases.
    s1 = g1 / np.sqrt(v1 + np.float32(EPS))
    w1f = (W1 * s1[None, :]).astype(np.float32)
    c1 = b1 * s1 + be1 - m1 * s1
    s2 = g2 / np.sqrt(v2 + np.float32(EPS))
    w2f = (W2 * s2[None, :]).astype(np.float32)
    c2 = b2 * s2 + be2 - m2 * s2

    # Remap W1 rows k=f*16+d to padded feature rows g=f*17+d (slot j=16 is
    # the emb1 value; its W1 row is zero).
    kk = np.arange(F * D)
    g_of_k = (kk // D) * TR + (kk % D)
    w1p = np.zeros((NF, H1), np.float32)
    w1p[g_of_k] = w1f

    gg = np.arange(NF)
    jj = gg % TR
    amat = np.zeros((NF, TS), np.float32)
    amat[gg[jj < D], jj[jj < D]] = 1.0       # s_d selectors
    amat[gg[jj == D], 32] = 1.0              # fm first-order selector (row 32)
    umat = np.zeros((NF, 1), np.float32)
    umat[jj < D, 0] = -0.5                   # -0.5 * sum_d sum_f e2^2
    wfin = np.zeros((TS, 1), np.float32)
    wfin[:D, 0] = 0.5                         # 0.5 * sum_d s_d^2 ...
    wfin[32, 0] = 1.0                         # ... + fm1

    # Pack all matmul weights into one [128, WRC] f32r tensor (one DMA).
    wpack_r = np.zeros((P, WRC), np.float32)
    for c in range(4):
        ch = FCH[c]
        r0 = c * P
        wpack_r[:ch, WC_W1 + c * H1: WC_W1 + (c + 1) * H1] = w1p[r0:r0 + ch]
        wpack_r[:ch, WC_A + c * TS: WC_A + (c + 1) * TS] = amat[r0:r0 + ch]
        wpack_r[:ch, WC_U + c] = umat[r0:r0 + ch, 0]
    for k in range(2):
        wpack_r[:, WC_W2 + k * H2: WC_W2 + (k + 1) * H2] = w2f[k * P:(k + 1) * P]
    wpack_r[:, WC_W3] = W3.reshape(H2)
    wpack_r[:TS, WC_FIN] = wfin[:, 0]
    wpack_r = wpack_r.astype(np_bf16)

    # Biases (f32): cols 0-1 = c1 per m-chunk, col 2 = c2, col 3 row 0 = b3.
    wpack_f = np.zeros((P, 4), np.float32)
    wpack_f[:, 0:2] = c1.reshape(H1 // P, P).T
    wpack_f[:, 2] = c2
    wpack_f[0, 3] = b3.reshape(-1)[0]

    # Global gather row ids; per-core SBUF layout [128, BT*F] with
    # col t*F+f holding batch row t*128+p.
    idx_g = X_sparse.astype(np.int32) + (np.arange(F, dtype=np.int32) * V)[None, :]

    in_maps = []
    for i in range(NCORES):
        gi = idx_g[i * NB:(i + 1) * NB].reshape(BT, P, F)
        idx_sb = np.ascontiguousarray(gi.transpose(1, 0, 2).reshape(P, BT * F))
        in_maps.append(dict(
            idx=idx_sb,
            table=table,
            wpack_r=wpack_r,
            wpack_f=wpack_f,
        ))

    nc = _get_nc()
    res = run_bass_kernel_spmd(
        nc, in_maps, core_ids=list(range(NCORES)), trace=TRACE
    )
    LAST_RESULTS = res

    out = np.empty((B, 1), np.float32)
    for i in range(NCORES):
        out[i * NB:(i + 1) * NB, 0] = np.asarray(res.results[i]["out"]).reshape(NB)
    return out

